# revision 1
# baseline (speedup 1.0000x reference)
"""Masked cross-attention kernel for Trainium2 (8 NeuronCores, SPMD).

Problem: B=16 batches of softmax(mask(Q@K^T/sqrt(D)))@V with
Lq=Lk=2048, D=DV=256.  The reference zeroes masked scores (NOT -inf)
before the softmax, so masked keys still contribute exp(0)=1 to the
denominator and weight 1/denom on V rows.

Strategy (all host prep is exact):
  * Zero K rows at k >= valid_length[b] on the host.  Then Q @ K^T is
    *exactly* 0.0 at masked positions - identical to the reference's
    jnp.where - and no mask tensor is needed on-device.
  * Pre-transpose Q and K to [D, L] layout on the host so both matmul
    operands stream naturally (contraction on the partition dim).
  * Append a ones-column to V.  P @ [V | 1] then yields the softmax
    denominator as output column 256 for free.
  * bf16 matmul inputs (fp32 PSUM accumulate), fp32 softmax math.
  * All per-batch inputs are packed host-side into ONE blob tensor
    [128 partitions x cols] and loaded in 3 big segment DMAs (a single
    DMA fans out across all 16 SDMA engines; many small DMAs each pay
    a ~2us completion latency and fair-share the engines).

Per core: 2 batches.  Per batch, for each 512-wide q tile:
  stage 1: S^T[k,q] tiles in PSUM (Kt.T @ Qt), exp via ScalarE
           (scale=1/16 folded in) -> P^T bf16 in SBUF
  stage 2: O[q,v] = (P^T).T @ [V|1] accumulated over k chunks in PSUM;
           divide by column 256 (DVE reciprocal + per-partition mul).
Stage 1 of q-tile i+1 is emitted before stage 2 of q-tile i so the PE
never stalls on the ScalarE exp chain.
"""

import numpy as np
import ml_dtypes

import concourse.bass as bass
import concourse.mybir as mybir
import concourse.tile as tile
from concourse import bacc
from concourse.bass_utils import run_bass_kernel_spmd

B, LQ, LK, D, DV = 16, 2048, 2048, 256, 256
N_CORES = 8
BPC = B // N_CORES  # batches per core

QT = 512            # q-tile width (stage-1 moving free dim)
NQT = LQ // QT      # 4
KT = 128            # k-tile (partition dim of S^T)
NKT = LK // KT      # 16
KG = 2              # k-tiles per PSUM/exp group
NKG = NKT // KG     # 8
NDC = D // 128      # contraction chunks (2)
QS = 128            # q-subtile for stage 2
NQS = QT // QS      # 4
VF = DV + 1         # 257: V plus the ones column
WARMUP_MMS = 11     # HAM warm-up matmuls bridging the initial DMA wait

# Blob column layout (per partition, bf16).  Segments sized so the
# latency-critical first working set (kt + qt0) splits evenly across
# the two independent HWDGE rings (sync + scalar), which run FIFO-serial
# per ring at ~170GB/s each:
#   seg A1: kt[c0,k0:8] 1024 | kt[c1,k0:8] 1024 | qt0 1024 -> 3072
#           (everything stage-1 groups g0-g3 of q-tile 0 need)
#   seg A2: kt[c0,k8:16] 1024 | kt[c1,k8:16] 1024          -> 2048
#   seg B1: v1a 8*VF=2056 ; seg B2: qt1 1024
#   seg C:  qt2 1024   | v1b 2056      -> 3080
#   seg D:  qt3 1024                   -> 1024
HK = LK // 2                         # 1024 kt columns per c per segment
SEG_A1 = NDC * HK + NDC * QT         # 3072
SEG_A2 = NDC * HK                    # 2048
SEG_B1 = (NKT // 2) * VF             # 2056 (v1a)
SEG_B2 = NDC * QT                    # 1024 (qt1)
SEG_C = NDC * QT + (NKT // 2) * VF   # 3080
SEG_D = NDC * QT                     # 1024
BLOB = SEG_A1 + SEG_A2 + SEG_B1 + SEG_B2 + SEG_C + SEG_D  # 12304

_BF16 = mybir.dt.bfloat16
_F32 = mybir.dt.float32

_NC_CACHE = {}


def _build_nc():
    nc = bacc.Bacc("TRN2", target_bir_lowering=False, debug=False,
                   num_devices=N_CORES)

    blob_d = nc.declare_dram_parameter("blob", [BPC, 128 * BLOB], _BF16,
                                       isOutput=False)
    out_d = nc.declare_dram_parameter("out", [BPC, LQ, DV], _F32,
                                      isOutput=True)

    with tile.TileContext(nc) as tc:
        with (
            tc.tile_pool(name="seg", bufs=2) as seg_pool,
            tc.tile_pool(name="p", bufs=2) as p_pool,
            tc.tile_pool(name="osb", bufs=4) as o_pool,
            tc.tile_pool(name="small", bufs=8) as small_pool,
            tc.tile_pool(name="ps_s", bufs=2, space="PSUM") as ps_s,
            tc.tile_pool(name="ps_o", bufs=4, space="PSUM") as ps_o,
        ):
            SEG_SIZES = [SEG_A1, SEG_A2, SEG_B1, SEG_B2, SEG_C, SEG_D]
            SEG_TBL = []
            _off = 0
            for _n in SEG_SIZES:
                SEG_TBL.append((_off, _n))
                _off += _n

            def load_batch(b):
                segs = []
                # Each segment is a fully CONTIGUOUS block in DRAM
                # (partition-major within the segment), so the DMA is a
                # pure sequential HBM read - max fan-out and HBM
                # efficiency.  All on the sync ring, FIFO in deadline
                # order.
                for si, (lo, n) in enumerate(SEG_TBL):
                    t = seg_pool.tile([128, n], _BF16, tag=f"seg{si}",
                                      name=f"seg{si}_b{b}")
                    src = blob_d[b, 128 * lo:128 * (lo + n)].rearrange(
                        "(p n) -> p n", p=128)
                    nc.sync.dma_start(out=t, in_=src)
                    segs.append(t)
                return segs

            def kt_slice(segs, c, kj):
                h, ko = kj // (NKT // 2), kj % (NKT // 2)
                o = c * HK + ko * KT
                return segs[h][:, o:o + KT]

            def qt_slice(segs, qi, c):
                if qi == 0:
                    return segs[0][:, NDC * HK + c * QT:NDC * HK + (c + 1) * QT]
                offs = [(3, 0), (4, 0), (5, 0)]
                si, o = offs[qi - 1]
                return segs[si][:, o + c * QT:o + (c + 1) * QT]

            def v1_slice(segs, kj):
                if kj < NKT // 2:
                    return segs[2][:, kj * VF:(kj + 1) * VF]
                kj -= NKT // 2
                return segs[4][:, NDC * QT + kj * VF:NDC * QT + (kj + 1) * VF]

            def stage1(segs, qi):
                """S^T = Kt.T @ Qt for one 512-wide q tile; exp -> P^T bf16."""
                p_sb = p_pool.tile([128, NKT * QT], _BF16, tag="p")
                for g in range(NKG):
                    ps = ps_s.tile([128, KG * QT], _F32, tag="s")
                    for h in range(KG):
                        kj = g * KG + h
                        for c in range(NDC):
                            nc.tensor.matmul(
                                ps[:, h * QT:(h + 1) * QT],
                                lhsT=kt_slice(segs, c, kj),
                                rhs=qt_slice(segs, qi, c),
                                start=(c == 0),
                                stop=(c == NDC - 1),
                            )
                    nc.scalar.activation(
                        out=p_sb[:, g * KG * QT:(g + 1) * KG * QT], in_=ps,
                        func=mybir.ActivationFunctionType.Exp,
                        scale=1.0 / 16.0)
                return p_sb

            def stage2(segs, b, qi, p_sb):
                """O = P @ [V|1]; normalize by the ones column; DMA out."""
                for s in range(NQS):
                    o_ps = ps_o.tile([128, VF], _F32, tag="o")
                    for kj in range(NKT):
                        nc.tensor.matmul(
                            o_ps,
                            lhsT=p_sb[:, kj * QT + s * QS:kj * QT + (s + 1) * QS],
                            rhs=v1_slice(segs, kj),
                            start=(kj == 0), stop=(kj == NKT - 1),
                        )
                    recip = small_pool.tile([128, 1], _F32, tag="r")
                    nc.vector.reciprocal(out=recip, in_=o_ps[:, DV:DV + 1])
                    o_sb = o_pool.tile([128, DV], _F32, tag="o_sb")
                    nc.vector.tensor_scalar_mul(
                        out=o_sb, in0=o_ps[:, :DV], scalar1=recip)
                    q0 = qi * QT + s * QS
                    nc.sync.dma_start(out=out_d[b, q0:q0 + QS, :], in_=o_sb)

            # HAM warm-up: matmuls on an UNINITIALIZED tile (no producer ->
            # no waits -> PE starts right after its preamble) into a
            # throwaway PSUM group.  The tiny DVE read keeps DCE away.
            warm = small_pool.tile([128, QT], _BF16, tag="warm")
            wps = ps_o.tile([128, QT], _F32, tag="o", name="warm_ps")
            for w in range(WARMUP_MMS):
                nc.tensor.matmul(wps, lhsT=warm[:, :128], rhs=warm,
                                 start=(w == 0), stop=(w == WARMUP_MMS - 1))
            nc.vector.tensor_copy(out=warm[:, 0:1], in_=wps[:, 0:1])

            states = [load_batch(b) for b in range(BPC)]
            work = [(b, qi) for b in range(BPC) for qi in range(NQT)]
            pending = None  # (segs, b, qi, p_sb)
            for b, qi in work:
                p_sb = stage1(states[b], qi)
                if pending is not None:
                    stage2(*pending)
                pending = (states[b], b, qi, p_sb)
            stage2(*pending)

    nc.compile()
    return nc


def _get_nc():
    if "nc" not in _NC_CACHE:
        _NC_CACHE["nc"] = _build_nc()
    return _NC_CACHE["nc"]


def _prepare(query, key, value, valid_length):
    query = np.asarray(query, dtype=np.float32)
    key = np.asarray(key, dtype=np.float32)
    value = np.asarray(value, dtype=np.float32)
    valid_length = np.asarray(valid_length)

    kz = key.copy()
    for b in range(B):
        kz[b, int(valid_length[b]):, :] = 0.0

    bf16 = ml_dtypes.bfloat16
    # ktc[c][b, p, k] = Kz[b, k, c*128+p]
    karr = kz.transpose(0, 2, 1).reshape(B, NDC, 128, LK)  # [B, c, p, k]
    # qc[b, p, c, q] = Q[b, q, c*128+p]
    qarr = query.transpose(0, 2, 1).reshape(B, NDC, 128, LQ) \
        .transpose(0, 2, 1, 3)  # [B, 128, NDC, LQ]
    qts = [qarr[:, :, :, qi * QT:(qi + 1) * QT].reshape(B, 128, NDC * QT)
           for qi in range(NQT)]
    # qt0 split per c-chunk for the A segments
    qt0c = [qarr[:, :, c, 0:QT] for c in range(NDC)]  # [B, 128, QT] each
    # v1h[b, p, t*VF + v] = [V|1][b, (8h+t)*128+p, v]
    v1 = np.concatenate(
        [value, np.ones((B, LK, 1), np.float32)], axis=-1)  # [B, LK, VF]
    v1arr = v1.reshape(B, NKT, 128, VF).transpose(0, 2, 1, 3)  # [B,128,NKT,VF]
    v1a = v1arr[:, :, 0:NKT // 2, :].reshape(B, 128, (NKT // 2) * VF)
    v1b = v1arr[:, :, NKT // 2:NKT, :].reshape(B, 128, (NKT // 2) * VF)

    seg_a1 = np.concatenate(
        [karr[:, 0, :, 0:HK], karr[:, 1, :, 0:HK], qt0c[0], qt0c[1]], axis=2)
    seg_a2 = np.concatenate(
        [karr[:, 0, :, HK:LK], karr[:, 1, :, HK:LK]], axis=2)
    seg_b1 = v1a
    seg_b2 = qts[1]
    seg_c = np.concatenate([qts[2], v1b], axis=2)
    seg_d = qts[3]
    # Flatten each segment partition-major so it is contiguous in DRAM.
    blob = np.concatenate(
        [s.reshape(B, -1) for s in (seg_a1, seg_a2, seg_b1, seg_b2, seg_c, seg_d)],
        axis=1)
    assert blob.shape == (B, 128 * BLOB)
    return np.ascontiguousarray(blob).astype(bf16)


def _run(inputs, trace=False):
    blob = _prepare(**inputs)
    in_maps = [{"blob": blob[c * BPC:(c + 1) * BPC]} for c in range(N_CORES)]
    nc = _get_nc()
    res = run_bass_kernel_spmd(nc, in_maps, core_ids=list(range(N_CORES)),
                               trace=trace)
    out = np.empty((B, LQ, DV), np.float32)
    for c in range(N_CORES):
        out[c * BPC:(c + 1) * BPC] = res.results[c]["out"]
    return out, res


def kernel(query, key, value, valid_length):
    out, _ = _run(dict(query=query, key=key, value=value,
                       valid_length=valid_length))
    return out



# revision 8
# speedup vs baseline: 1.5793x; 1.5793x over previous
"""Masked cross-attention kernel for Trainium2 (8 NeuronCores, SPMD).

Problem: B=16 batches of softmax(mask(Q@K^T/sqrt(D)))@V with
Lq=Lk=2048, D=DV=256.  The reference zeroes masked scores (NOT -inf)
before the softmax, so masked keys contribute exp(0)=1 to the
denominator and weight 1/denom on V rows.

KEY OPTIMIZATION vs the dense version: for keys k >= valid_length the
score is *exactly* 0 (K rows are host-zeroed), so every masked key
contributes exp(0)=1 * v_k to the numerator and 1 to the denominator.
A whole k-tile range [t*128, 2048) therefore collapses to a single
per-batch correction vector  corr = sum_{k>=t*128} [V|1][k]  that the
HOST precomputes exactly and FOLDS into the last (masked) V rows of
the computed range - those rows have attention weight exactly 1, so
adding corr there is exact and costs zero device instructions.  Only
ceil(valid_length/128) k-tiles of real matmul work remain per batch
(~half, for uniform valid_length).

SPMD constraint: one program runs on all 8 cores, so per-core work
must be structurally identical.  Work units are 64 jobs = (batch,
512-wide q tile) each needing r_b k-tiles.  Jobs are sorted by r
descending and dealt 8-per-slot into 8 slots; slot j executes
t_j = max r in slot on every core (padding tiles have zeroed K ->
exp(0)=1 -> exact masked behavior).  Sum t_j is within ~6% of the
ideal balance.  Slots execute smallest-first: the first DMA is tiny
(fast start) and exp latency always hides behind the next slot's
larger stage-1.

Per slot, per core: stage 1: S^T[k,q] in PSUM (Kt.T @ Qt), exp via
ScalarE (scale=1/16 folded) -> P^T bf16; stage 2: O[q,v] = (P^T).T @
[V|1] accumulated over k tiles, divide by column 256.  Stage 1 of
slot e+1 is emitted before stage 2 of slot e so the PE never waits on
the ScalarE exp chain.

DMA: K+Q segments on the sync HWDGE ring, V segments + fp16 outputs
on the scalar ring (~170 GB/s each, FIFO-serial per ring).
"""

import numpy as np
import ml_dtypes

import concourse.bass as bass
import concourse.mybir as mybir
import concourse.tile as tile
from concourse import bacc
from concourse.bass_utils import run_bass_kernel_spmd

B, LQ, LK, D, DV = 16, 2048, 2048, 256, 256
N_CORES = 8

QT = 512            # q-tile width (stage-1 moving free dim)
NQT = LQ // QT      # 4 q tiles per batch
KT = 128            # k-tile (partition dim of S^T)
NKT = LK // KT      # 16 k tiles max
NDC = D // 128      # contraction chunks (2)
QS = 128            # q-subtile for stage 2
NQS = QT // QS      # 4
VF = DV + 1         # 257: V plus the ones column
NSLOT = 8           # jobs per core == slots
WARMUP_MMS = 8      # HAM warm-up matmuls bridging the initial DMA wait

_BF16 = mybir.dt.bfloat16
_F16 = mybir.dt.float16
_F32 = mybir.dt.float32

_NC_CACHE = {}


def _schedule(vl):
    """Compute per-batch tile counts, the common slot profile (ascending
    execution order) and the per-core job assignment."""
    r = []
    for b in range(B):
        v = int(vl[b])
        rb = max(1, -(-v // KT))          # ceil(v/128), min 1
        # Guarantee >=2 masked rows inside the computed range whenever a
        # suffix correction exists (rb < NKT), so the correction can be
        # folded into masked V rows as a bf16 hi/lo pair.
        if rb < NKT and rb * KT - v < 2:
            rb += 1
        r.append(rb)
    jobs = [(r[b], b, qi) for b in range(B) for qi in range(NQT)]
    jobs.sort(key=lambda x: (-x[0], x[1], x[2]))
    # slot j (descending) = jobs[8j:8j+8]; execution order is ascending t.
    profile = []
    assign = [[None] * NSLOT for _ in range(N_CORES)]
    for j in range(NSLOT):
        grp = jobs[N_CORES * j:N_CORES * (j + 1)]
        e = NSLOT - 1 - j                  # execution index (ascending t)
        profile_t = max(1, grp[0][0])
        profile.append((e, profile_t))
        for c in range(N_CORES):
            assign[c][e] = (grp[c][1], grp[c][2])
    prof = [0] * NSLOT
    for e, t in profile:
        prof[e] = t
    return tuple(prof), assign, r


def _layout(profile):
    """Blob column layout (per 128-partition, bf16 elements).

    Chunks in DMA order: per slot e: K_e ([pair-major: c0 t2p | c0 t2p+1 |
    c1 t2p | c1 t2p+1] x npair, zero-padded to whole pairs) and
    Q_e ([c0 512 | c1 512]); then all V_e ([t*257], k-tile major).
    K_0 is split into head (<=2 pairs) + tail for a fast first DMA.
    Returns (chunk list [(name, ncols)], total cols).
    """
    chunks = []
    for e, t in enumerate(profile):
        npair = -(-t // 2)
        if e == 0 and npair > 2:
            chunks.append((f"k{e}h", 2 * 512))
            chunks.append((f"q{e}", NDC * QT))
            chunks.append((f"k{e}t", (npair - 2) * 512))
        else:
            chunks.append((f"k{e}h", npair * 512))
            chunks.append((f"q{e}", NDC * QT))
    for e, t in enumerate(profile):
        chunks.append((f"v{e}", t * VF))
    tot = sum(n for _, n in chunks)
    return chunks, tot


def _segments(profile):
    """DMA segment grouping: list of (ring, [chunk names]).  Must match
    between host packing and device loads (a multi-chunk segment is
    stored partition-major over the WHOLE segment)."""
    segs = []
    npair0 = -(-profile[0] // 2)
    for e in range(len(profile)):
        if e == 0 and npair0 > 2:
            segs.append(("sync", [f"k{e}h", f"q{e}"]))
            segs.append(("sync", [f"k{e}t"]))
        else:
            segs.append(("sync", [f"k{e}h", f"q{e}"]))
    for e in range(len(profile)):
        segs.append(("scalar", [f"v{e}"]))
    return segs


def _build_nc(profile):
    chunks, tot = _layout(profile)
    off = {}
    _o = 0
    for name, n in chunks:
        off[name] = (_o, n)
        _o += n

    nc = bacc.Bacc("TRN2", target_bir_lowering=False, debug=False,
                   num_devices=N_CORES)
    blob_d = nc.declare_dram_parameter("blob", [128 * tot], _BF16,
                                       isOutput=False)
    out_d = nc.declare_dram_parameter("out", [NSLOT, QT, DV], _F16,
                                      isOutput=True)
    tmax = max(profile)

    with tile.TileContext(nc) as tc:
        with (
            tc.tile_pool(name="seg", bufs=1) as seg_pool,
            tc.tile_pool(name="p", bufs=2) as p_pool,
            tc.tile_pool(name="osb", bufs=4) as o_pool,
            tc.tile_pool(name="small", bufs=8) as small_pool,
            tc.tile_pool(name="ps_s", bufs=2, space="PSUM") as ps_s,
            tc.tile_pool(name="ps_o", bufs=4, space="PSUM") as ps_o,
        ):
            tiles = {}

            def load(names, engine):
                """One DMA for a run of consecutive chunks -> one SBUF tile."""
                lo = off[names[0]][0]
                n = sum(off[nm][1] for nm in names)
                t = seg_pool.tile([128, n], _BF16, tag="+".join(names))
                src = blob_d[128 * lo:128 * (lo + n)].rearrange(
                    "(p n) -> p n", p=128)
                engine.dma_start(out=t, in_=src)
                for nm in names:
                    tiles[nm] = (t, off[nm][0] - lo)
                return t

            def chunk_slice(nm, a, b):
                t, o = tiles[nm]
                return t[:, o + a:o + b]

            def kt_slice(e, c, kj):
                pair, half = kj // 2, kj % 2
                col = pair * 512 + c * 256 + half * 128
                hname = f"k{e}h"
                hcols = off[hname][1]
                if col < hcols:
                    return chunk_slice(hname, col, col + 128)
                return chunk_slice(f"k{e}t", col - hcols, col - hcols + 128)

            def qt_slice(e, c):
                return chunk_slice(f"q{e}", c * QT, (c + 1) * QT)

            def v1_slice(e, kj):
                return chunk_slice(f"v{e}", kj * VF, (kj + 1) * VF)

            # ---- input DMAs: K+Q on sync ring, V on scalar ring ----
            for ring, names in _segments(profile):
                if ring == "sync":
                    load(names, nc.sync)
            for ring, names in _segments(profile):
                if ring == "scalar":
                    load(names, nc.scalar)

            def stage1(e):
                """S^T = Kt.T @ Qt for one job; exp -> P^T bf16."""
                t = profile[e]
                npair = -(-t // 2)
                p_sb = p_pool.tile([128, tmax * QT], _BF16, tag="p")
                for g in range(npair):
                    w = 2 if 2 * g + 1 < t else 1
                    ps = ps_s.tile([128, 2 * QT], _F32, tag="s")
                    for h in range(w):
                        kj = 2 * g + h
                        for c in range(NDC):
                            nc.tensor.matmul(
                                ps[:, h * QT:(h + 1) * QT],
                                lhsT=kt_slice(e, c, kj),
                                rhs=qt_slice(e, c),
                                start=(c == 0),
                                stop=(c == NDC - 1),
                            )
                    nc.scalar.activation(
                        out=p_sb[:, g * 2 * QT:g * 2 * QT + w * QT],
                        in_=ps[:, :w * QT],
                        func=mybir.ActivationFunctionType.Exp,
                        scale=1.0 / 16.0)
                return p_sb

            def stage2(e, p_sb):
                """O = P @ [V|1]; normalize by the ones column; DMA out."""
                t = profile[e]
                for s in range(NQS):
                    o_ps = ps_o.tile([128, VF], _F32, tag="o")
                    for kj in range(t):
                        nc.tensor.matmul(
                            o_ps,
                            lhsT=p_sb[:, kj * QT + s * QS:
                                      kj * QT + (s + 1) * QS],
                            rhs=v1_slice(e, kj),
                            start=(kj == 0), stop=(kj == t - 1),
                        )
                    recip = small_pool.tile([128, 1], _F32, tag="r")
                    nc.vector.reciprocal(out=recip, in_=o_ps[:, DV:DV + 1])
                    o_sb = o_pool.tile([128, DV], _F16, tag="o_sb")
                    nc.vector.tensor_scalar_mul(
                        out=o_sb, in0=o_ps[:, :DV], scalar1=recip)
                    nc.scalar.dma_start(
                        out=out_d[e, s * QS:(s + 1) * QS, :], in_=o_sb)

            # HAM warm-up: matmuls on an UNINITIALIZED tile (no producer ->
            # no waits -> PE starts right after its preamble) into a
            # throwaway PSUM group.  The tiny DVE read keeps DCE away.
            warm = small_pool.tile([128, QT], _BF16, tag="warm")
            wps = ps_o.tile([128, QT], _F32, tag="o", name="warm_ps")
            for w in range(WARMUP_MMS):
                nc.tensor.matmul(wps, lhsT=warm[:, :128], rhs=warm,
                                 start=(w == 0), stop=(w == WARMUP_MMS - 1))
            nc.vector.tensor_copy(out=warm[:, 0:1], in_=wps[:, 0:1])

            pending = None
            for e in range(NSLOT):
                p_sb = stage1(e)
                if pending is not None:
                    stage2(*pending)
                pending = (e, p_sb)
            stage2(*pending)

    nc.compile()
    return nc


def _get_nc(profile):
    if profile not in _NC_CACHE:
        _NC_CACHE[profile] = _build_nc(profile)
    return _NC_CACHE[profile]


def _prepare(query, key, value, valid_length):
    query = np.asarray(query, dtype=np.float32)
    key = np.asarray(key, dtype=np.float32)
    value = np.asarray(value, dtype=np.float32)
    vl = np.asarray(valid_length).astype(np.int64)

    profile, assign, r = _schedule(vl)
    chunks, tot = _layout(profile)

    bf16 = ml_dtypes.bfloat16

    kz = key.copy()
    for b in range(B):
        kz[b, int(vl[b]):, :] = 0.0
    # kzT[b] = [c, 128, LK];  qT[b] = [c, 128, LQ]
    kzT = kz.transpose(0, 2, 1).reshape(B, NDC, 128, LK)
    qT = query.transpose(0, 2, 1).reshape(B, NDC, 128, LQ)
    v1 = np.concatenate(
        [value, np.ones((B, LK, 1), np.float32)], axis=-1)  # [B, LK, VF]
    # suffix sums at tile boundaries: suf[b, m] = sum_{k >= m*128} v1[b, k]
    blk = v1.reshape(B, NKT, KT, VF).sum(axis=2)            # [B, 16, VF]
    suf = np.zeros((B, NKT + 1, VF), np.float32)
    suf[:, :NKT] = blk[:, ::-1].cumsum(axis=1)[:, ::-1]

    def k_chunk(b, t):
        npair = -(-t // 2)
        kp = np.zeros((NDC, 128, npair * 256), np.float32)
        kp[:, :, :t * KT] = kzT[b][:, :, :t * KT]
        # pair-major: [pair, c, 256]
        arr = kp.reshape(NDC, 128, npair, 256).transpose(1, 2, 0, 3)
        return arr.reshape(128, npair * 512)

    def q_chunk(b, qi):
        return qT[b][:, :, qi * QT:(qi + 1) * QT] \
            .transpose(1, 0, 2).reshape(128, NDC * QT)

    def v_chunk(b, t):
        vj = v1[b, :t * KT].copy()                          # [t*128, VF]
        if t < NKT:
            # fold the exact masked-suffix correction into the last two
            # rows of the computed range (both masked -> weight exactly 1)
            T = vj[-1] + vj[-2] + suf[b, t]
            hi = T.astype(bf16).astype(np.float32)
            vj[-1] = hi
            vj[-2] = T - hi
        return vj.reshape(t, 128, VF).transpose(1, 0, 2).reshape(128, t * VF)

    blobs = []
    for c in range(N_CORES):
        parts = {}
        for e, t in enumerate(profile):
            b, qi = assign[c][e]
            kc = k_chunk(b, t)
            if f"k{e}t" in dict(chunks):
                parts[f"k{e}h"] = kc[:, :2 * 512]
                parts[f"k{e}t"] = kc[:, 2 * 512:]
            else:
                parts[f"k{e}h"] = kc
            parts[f"q{e}"] = q_chunk(b, qi)
            parts[f"v{e}"] = v_chunk(b, t)
        flat_segs = []
        for _ring, names in _segments(profile):
            seg = (np.concatenate([parts[nm] for nm in names], axis=1)
                   if len(names) > 1 else parts[names[0]])
            flat_segs.append(seg.astype(bf16).reshape(-1))
        flat = np.concatenate(flat_segs)
        assert flat.shape == (128 * tot,)
        blobs.append(flat)
    return blobs, profile, assign


def _run(inputs, trace=False):
    blobs, profile, assign = _prepare(**inputs)
    in_maps = [{"blob": blobs[c]} for c in range(N_CORES)]
    nc = _get_nc(profile)
    res = run_bass_kernel_spmd(nc, in_maps, core_ids=list(range(N_CORES)),
                               trace=trace)
    out = np.empty((B, LQ, DV), np.float32)
    for c in range(N_CORES):
        r_c = np.asarray(res.results[c]["out"], dtype=np.float32)
        for e in range(NSLOT):
            b, qi = assign[c][e]
            out[b, qi * QT:(qi + 1) * QT] = r_c[e]
    return out, res


def kernel(query, key, value, valid_length):
    out, _ = _run(dict(query=query, key=key, value=value,
                       valid_length=valid_length))
    return out


# revision 10
# speedup vs baseline: 1.6853x; 1.0671x over previous
"""Masked cross-attention kernel for Trainium2 (8 NeuronCores, SPMD).

Problem: B=16 batches of softmax(mask(Q@K^T/sqrt(D)))@V with
Lq=Lk=2048, D=DV=256.  The reference zeroes masked scores (NOT -inf)
before the softmax, so masked keys contribute exp(0)=1 to the
denominator and weight 1/denom on V rows.

KEY OPTIMIZATION vs the dense version: for keys k >= valid_length the
score is *exactly* 0 (K rows are host-zeroed), so every masked key
contributes exp(0)=1 * v_k to the numerator and 1 to the denominator.
A whole k-tile range [t*128, 2048) therefore collapses to a single
per-batch correction vector  corr = sum_{k>=t*128} [V|1][k]  that the
HOST precomputes exactly and FOLDS into the last (masked) V rows of
the computed range - those rows have attention weight exactly 1, so
adding corr there is exact and costs zero device instructions.  Only
ceil(valid_length/128) k-tiles of real matmul work remain per batch
(~half, for uniform valid_length).

SPMD constraint: one program runs on all 8 cores, so per-core work
must be structurally identical.  Work units are 64 jobs = (batch,
512-wide q tile) each needing r_b k-tiles.  Jobs are sorted by r
descending and dealt 8-per-slot into 8 slots; slot j executes
t_j = max r in slot on every core (padding tiles have zeroed K ->
exp(0)=1 -> exact masked behavior).  Sum t_j is within ~6% of the
ideal balance.  Slots execute smallest-first so the first DMA is
tiny (fast start).

Per slot, per core: stage 1: S^T[k,q] in PSUM (Kt.T @ Qt), exp via
ScalarE (scale=1/16 folded) -> P^T bf16; stage 2: O[q,v] = (P^T).T @
[V|1] accumulated over k tiles, divide by column 256.  Stage-2
subtile chains of slot e-1 are interleaved BETWEEN stage-1 exp groups
of slot e so the PE always has queued work while ScalarE's exp chain
(the 2nd-busiest engine) catches up.

DMA: segments are enumerated in need order and greedily split across
the two HWDGE rings (sync + scalar engines, ~170 GB/s each,
FIFO-serial per ring) so aggregate streaming tracks consumption.
Outputs are batched one DMA per job ([128, 4*256] fp16, strided DRAM
view) to cut descriptor-write cost on the issuing engines.
"""

import numpy as np
import ml_dtypes

import concourse.bass as bass
import concourse.mybir as mybir
import concourse.tile as tile
from concourse import bacc
from concourse.bass_utils import run_bass_kernel_spmd

B, LQ, LK, D, DV = 16, 2048, 2048, 256, 256
N_CORES = 8

QT = 512            # q-tile width (stage-1 moving free dim)
NQT = LQ // QT      # 4 q tiles per batch
KT = 128            # k-tile (partition dim of S^T)
NKT = LK // KT      # 16 k tiles max
NDC = D // 128      # contraction chunks (2)
QS = 128            # q-subtile for stage 2
NQS = QT // QS      # 4
VF = DV + 1         # 257: V plus the ones column
NSLOT = 8           # jobs per core == slots
WARMUP_MMS = 8      # HAM warm-up matmuls bridging the initial DMA wait

_BF16 = mybir.dt.bfloat16
_F16 = mybir.dt.float16
_F32 = mybir.dt.float32

_NC_CACHE = {}


def _schedule(vl):
    """Per-batch tile counts, the common slot profile (ascending
    execution order) and the per-core job assignment."""
    r = []
    for b in range(B):
        v = int(vl[b])
        rb = max(1, -(-v // KT))          # ceil(v/128), min 1
        # Guarantee >=2 masked rows inside the computed range whenever a
        # suffix correction exists (rb < NKT), so the correction can be
        # folded into masked V rows as a bf16 hi/lo pair.
        if rb < NKT and rb * KT - v < 2:
            rb += 1
        r.append(rb)
    jobs = [(r[b], b, qi) for b in range(B) for qi in range(NQT)]
    jobs.sort(key=lambda x: (-x[0], x[1], x[2]))
    profile = [0] * NSLOT
    assign = [[None] * NSLOT for _ in range(N_CORES)]
    for j in range(NSLOT):
        grp = jobs[N_CORES * j:N_CORES * (j + 1)]
        e = NSLOT - 1 - j                  # execution index (ascending t)
        profile[e] = max(1, grp[0][0])
        for c in range(N_CORES):
            assign[c][e] = (grp[c][1], grp[c][2])
    return tuple(profile), assign, r


def _npair(t):
    return -(-t // 2)


def _chunk_cols(profile, nm):
    e = int(nm[1:-1]) if nm[-1] in "ht" else int(nm[1:])
    if nm[0] == "k":
        np_ = _npair(profile[e])
        if nm[-1] == "h":
            return 2 * 512 if (e == 0 and np_ > 2) else np_ * 512
        return (np_ - 2) * 512
    if nm[0] == "q":
        return NDC * QT
    return profile[e] * VF


def _segments(profile):
    """DMA segments in NEED order, greedily assigned to the two HWDGE
    rings by queued bytes.  The blob layout follows this order; a
    multi-chunk segment is stored partition-major over the WHOLE
    segment."""
    need = []
    for e in range(NSLOT):
        if e == 0 and _npair(profile[0]) > 2:
            need.append([f"k{e}h", f"q{e}"])
            need.append([f"k{e}t"])
        else:
            need.append([f"k{e}h", f"q{e}"])
        if e >= 2:
            need.append([f"v{e - 2}"])
    need.append([f"v{NSLOT - 2}"])
    need.append([f"v{NSLOT - 1}"])

    load = {"sync": 0, "scalar": 0}
    segs = []
    for names in need:
        ring = "sync" if load["sync"] <= load["scalar"] else "scalar"
        load[ring] += sum(_chunk_cols(profile, nm) for nm in names)
        segs.append((ring, names))
    return segs


def _layout(profile):
    """Blob offsets per chunk, in segment order."""
    segs = _segments(profile)
    off = {}
    o = 0
    for _ring, names in segs:
        for nm in names:
            n = _chunk_cols(profile, nm)
            off[nm] = (o, n)
            o += n
    return segs, off, o


def _build_nc(profile):
    segs, off, tot = _layout(profile)

    nc = bacc.Bacc("TRN2", target_bir_lowering=False, debug=False,
                   num_devices=N_CORES)
    blob_d = nc.declare_dram_parameter("blob", [128 * tot], _BF16,
                                       isOutput=False)
    out_d = nc.declare_dram_parameter("out", [NSLOT, QT, DV], _F16,
                                      isOutput=True)
    tmax = max(profile)

    with tile.TileContext(nc) as tc:
        with (
            tc.tile_pool(name="seg", bufs=1) as seg_pool,
            tc.tile_pool(name="p", bufs=2) as p_pool,
            tc.tile_pool(name="osb", bufs=3) as o_pool,
            tc.tile_pool(name="small", bufs=8) as small_pool,
            tc.tile_pool(name="ps_s", bufs=2, space="PSUM") as ps_s,
            tc.tile_pool(name="ps_o", bufs=4, space="PSUM") as ps_o,
        ):
            tiles = {}

            def load(names, engine):
                lo = off[names[0]][0]
                n = sum(off[nm][1] for nm in names)
                t = seg_pool.tile([128, n], _BF16, tag="+".join(names))
                src = blob_d[128 * lo:128 * (lo + n)].rearrange(
                    "(p n) -> p n", p=128)
                engine.dma_start(out=t, in_=src)
                for nm in names:
                    tiles[nm] = (t, off[nm][0] - lo)

            def chunk_slice(nm, a, b):
                t, o = tiles[nm]
                return t[:, o + a:o + b]

            def kt_slice(e, c, kj):
                pair, half = kj // 2, kj % 2
                col = pair * 512 + c * 256 + half * 128
                hcols = off[f"k{e}h"][1]
                if col < hcols:
                    return chunk_slice(f"k{e}h", col, col + 128)
                return chunk_slice(f"k{e}t", col - hcols, col - hcols + 128)

            def qt_slice(e, c):
                return chunk_slice(f"q{e}", c * QT, (c + 1) * QT)

            def v1_slice(e, kj):
                return chunk_slice(f"v{e}", kj * VF, (kj + 1) * VF)

            for ring, names in segs:
                load(names, nc.sync if ring == "sync" else nc.scalar)

            # HAM warm-up: matmuls on an UNINITIALIZED tile (no producer ->
            # no waits -> PE starts right after its preamble) into a
            # throwaway PSUM group.  The tiny DVE read keeps DCE away.
            warm = small_pool.tile([128, QT], _BF16, tag="warm")
            wps = ps_o.tile([128, QT], _F32, tag="o", name="warm_ps")
            for w in range(WARMUP_MMS):
                nc.tensor.matmul(wps, lhsT=warm[:, :128], rhs=warm,
                                 start=(w == 0), stop=(w == WARMUP_MMS - 1))
            nc.vector.tensor_copy(out=warm[:, 0:1], in_=wps[:, 0:1])

            out_engines = [nc.sync, nc.scalar]

            def s2_unit(e, p_sb, s, o_job):
                """One stage-2 q-subtile: accumulate over k tiles,
                normalize, and (on the last subtile) DMA the whole job."""
                t = profile[e]
                o_ps = ps_o.tile([128, VF], _F32, tag="o")
                for kj in range(t):
                    nc.tensor.matmul(
                        o_ps,
                        lhsT=p_sb[:, kj * QT + s * QS:kj * QT + (s + 1) * QS],
                        rhs=v1_slice(e, kj),
                        start=(kj == 0), stop=(kj == t - 1),
                    )
                recip = small_pool.tile([128, 1], _F32, tag="r")
                nc.vector.reciprocal(out=recip, in_=o_ps[:, DV:DV + 1])
                nc.vector.tensor_scalar_mul(
                    out=o_job[:, s * DV:(s + 1) * DV], in0=o_ps[:, :DV],
                    scalar1=recip)
                if s == NQS - 1:
                    dst = out_d[e].rearrange("(s p) v -> p s v", s=NQS, p=QS)
                    src = o_job.rearrange("p (s v) -> p s v", s=NQS)
                    out_engines[e % 2].dma_start(out=dst, in_=src)

            pending = None  # (e, p_sb, o_job, next_subtile)
            for e in range(NSLOT):
                t = profile[e]
                p_sb = p_pool.tile([128, tmax * QT], _BF16, tag="p")
                for g in range(_npair(t)):
                    w = 2 if 2 * g + 1 < t else 1
                    ps = ps_s.tile([128, 2 * QT], _F32, tag="s")
                    for h in range(w):
                        kj = 2 * g + h
                        for c in range(NDC):
                            nc.tensor.matmul(
                                ps[:, h * QT:(h + 1) * QT],
                                lhsT=kt_slice(e, c, kj),
                                rhs=qt_slice(e, c),
                                start=(c == 0),
                                stop=(c == NDC - 1),
                            )
                    nc.scalar.activation(
                        out=p_sb[:, g * 2 * QT:g * 2 * QT + w * QT],
                        in_=ps[:, :w * QT],
                        func=mybir.ActivationFunctionType.Exp,
                        scale=1.0 / 16.0)
                    # keep the PE fed while ScalarE works through exp:
                    # slot e-1's stage-2 subtile chains slot between groups
                    if pending is not None and g >= 1 and pending[3] < NQS:
                        ep, pp, oj, si = pending
                        s2_unit(ep, pp, si, oj)
                        pending = (ep, pp, oj, si + 1)
                if pending is not None:
                    ep, pp, oj, si = pending
                    for s in range(si, NQS):
                        s2_unit(ep, pp, s, oj)
                o_job = o_pool.tile([128, NQS * DV], _F16, tag="o_job")
                pending = (e, p_sb, o_job, 0)
            ep, pp, oj, si = pending
            for s in range(si, NQS):
                s2_unit(ep, pp, s, oj)

    nc.compile()
    return nc


def _get_nc(profile):
    if profile not in _NC_CACHE:
        _NC_CACHE[profile] = _build_nc(profile)
    return _NC_CACHE[profile]


def _prepare(query, key, value, valid_length):
    query = np.asarray(query, dtype=np.float32)
    key = np.asarray(key, dtype=np.float32)
    value = np.asarray(value, dtype=np.float32)
    vl = np.asarray(valid_length).astype(np.int64)

    profile, assign, r = _schedule(vl)
    segs, off, tot = _layout(profile)

    bf16 = ml_dtypes.bfloat16

    kz = key.copy()
    for b in range(B):
        kz[b, int(vl[b]):, :] = 0.0
    kzT = kz.transpose(0, 2, 1).reshape(B, NDC, 128, LK)
    qT = query.transpose(0, 2, 1).reshape(B, NDC, 128, LQ)
    v1 = np.concatenate(
        [value, np.ones((B, LK, 1), np.float32)], axis=-1)  # [B, LK, VF]
    # suffix sums at tile boundaries: suf[b, m] = sum_{k >= m*128} v1[b, k]
    blk = v1.reshape(B, NKT, KT, VF).sum(axis=2)            # [B, 16, VF]
    suf = np.zeros((B, NKT + 1, VF), np.float32)
    suf[:, :NKT] = blk[:, ::-1].cumsum(axis=1)[:, ::-1]

    def k_chunk(b, t):
        np_ = _npair(t)
        kp = np.zeros((NDC, 128, np_ * 256), np.float32)
        kp[:, :, :t * KT] = kzT[b][:, :, :t * KT]
        arr = kp.reshape(NDC, 128, np_, 256).transpose(1, 2, 0, 3)
        return arr.reshape(128, np_ * 512)

    def q_chunk(b, qi):
        return qT[b][:, :, qi * QT:(qi + 1) * QT] \
            .transpose(1, 0, 2).reshape(128, NDC * QT)

    def v_chunk(b, t):
        vj = v1[b, :t * KT].copy()                          # [t*128, VF]
        if t < NKT:
            # fold the exact masked-suffix correction into the last two
            # rows of the computed range (both masked -> weight exactly 1)
            T = vj[-1] + vj[-2] + suf[b, t]
            hi = T.astype(bf16).astype(np.float32)
            vj[-1] = hi
            vj[-2] = T - hi
        return vj.reshape(t, 128, VF).transpose(1, 0, 2).reshape(128, t * VF)

    blobs = []
    for c in range(N_CORES):
        parts = {}
        for e, t in enumerate(profile):
            b, qi = assign[c][e]
            kc = k_chunk(b, t)
            if f"k{e}t" in off:
                parts[f"k{e}h"] = kc[:, :2 * 512]
                parts[f"k{e}t"] = kc[:, 2 * 512:]
            else:
                parts[f"k{e}h"] = kc
            parts[f"q{e}"] = q_chunk(b, qi)
            parts[f"v{e}"] = v_chunk(b, t)
        flat_segs = []
        for _ring, names in segs:
            seg = (np.concatenate([parts[nm] for nm in names], axis=1)
                   if len(names) > 1 else parts[names[0]])
            flat_segs.append(seg.astype(bf16).reshape(-1))
        flat = np.concatenate(flat_segs)
        assert flat.shape == (128 * tot,)
        blobs.append(flat)
    return blobs, profile, assign


def _run(inputs, trace=False):
    blobs, profile, assign = _prepare(**inputs)
    in_maps = [{"blob": blobs[c]} for c in range(N_CORES)]
    nc = _get_nc(profile)
    res = run_bass_kernel_spmd(nc, in_maps, core_ids=list(range(N_CORES)),
                               trace=trace)
    out = np.empty((B, LQ, DV), np.float32)
    for c in range(N_CORES):
        r_c = np.asarray(res.results[c]["out"], dtype=np.float32)
        for e in range(NSLOT):
            b, qi = assign[c][e]
            out[b, qi * QT:(qi + 1) * QT] = r_c[e]
    return out, res


def kernel(query, key, value, valid_length):
    out, _ = _run(dict(query=query, key=key, value=value,
                       valid_length=valid_length))
    return out


# revision 13
# speedup vs baseline: 1.9418x; 1.1522x over previous
"""Masked cross-attention kernel for Trainium2 (8 NeuronCores, SPMD).

Problem: B=16 batches of softmax(mask(Q@K^T/sqrt(D)))@V with
Lq=Lk=2048, D=DV=256.  The reference zeroes masked scores (NOT -inf)
before the softmax, so masked keys contribute exp(0)=1 to the
denominator and weight 1/denom on V rows.

KEY OPTIMIZATION vs the dense version: for keys k >= valid_length the
score is *exactly* 0 (K rows are host-zeroed), so every masked key
contributes exp(0)=1 * v_k to the numerator and 1 to the denominator.
A whole k-tile range [t*128, 2048) therefore collapses to a single
per-batch correction vector  corr = sum_{k>=t*128} [V|1][k]  that the
HOST precomputes exactly and FOLDS into the last (masked) V rows of
the computed range - those rows have attention weight exactly 1, so
adding corr there is exact and costs zero device instructions.  Only
ceil(valid_length/128) k-tiles of real matmul work remain per batch
(~half, for uniform valid_length).

SPMD constraint: one program runs on all 8 cores, so per-core work
must be structurally identical.  Work units are 64 jobs = (batch,
512-wide q tile) each needing r_b k-tiles.  Jobs are sorted by r
descending and dealt 8-per-slot into 8 slots; slot j executes
t_j = max r in slot on every core (padding tiles have zeroed K ->
exp(0)=1 -> exact masked behavior).  Sum t_j is within ~6% of the
ideal balance.  Slots execute smallest-first so the first DMA is
tiny (fast start).

Per slot, per core: stage 1: S^T[k,q] in PSUM (Kt.T @ Qt), exp via
ScalarE (scale=1/16 folded) -> P^T bf16; stage 2: O[q,v] = (P^T).T @
[V|1] accumulated over k tiles, divide by column 256.  Stage-2
subtile chains of slot e-1 are interleaved BETWEEN stage-1 exp groups
of slot e so the PE always has queued work while ScalarE's exp chain
(the 2nd-busiest engine) catches up.

DMA: segments are enumerated in need order and greedily split across
the two HWDGE rings (sync + scalar engines, ~170 GB/s each,
FIFO-serial per ring) so aggregate streaming tracks consumption.
Outputs are batched one DMA per job ([128, 4*256] fp16, strided DRAM
view) to cut descriptor-write cost on the issuing engines.
"""

import numpy as np
import ml_dtypes

import concourse.bass as bass
import concourse.mybir as mybir
import concourse.tile as tile
from concourse import bacc
from concourse.bass_utils import run_bass_kernel_spmd

B, LQ, LK, D, DV = 16, 2048, 2048, 256, 256
N_CORES = 8

QT = 512            # q-tile width (stage-1 moving free dim)
NQT = LQ // QT      # 4 q tiles per batch
KT = 128            # k-tile (partition dim of S^T)
NKT = LK // KT      # 16 k tiles max
NDC = D // 128      # contraction chunks (2)
QS = 128            # q-subtile for stage 2
NQS = QT // QS      # 4
VF = DV + 1         # 257: V plus the ones column
NSLOT = 8           # jobs per core == slots
WARMUP_MMS = 8      # HAM warm-up matmuls bridging the initial DMA wait

_BF16 = mybir.dt.bfloat16
_F16 = mybir.dt.float16
_F32 = mybir.dt.float32

_NC_CACHE = {}


def _schedule(vl):
    """Per-batch tile counts, the common slot profile (ascending
    execution order) and the per-core job assignment."""
    r = []
    for b in range(B):
        v = int(vl[b])
        rb = max(1, -(-v // KT))          # ceil(v/128), min 1
        # Guarantee >=2 masked rows inside the computed range whenever a
        # suffix correction exists (rb < NKT), so the correction can be
        # folded into masked V rows as a bf16 hi/lo pair.
        if rb < NKT and rb * KT - v < 2:
            rb += 1
        r.append(rb)
    jobs = [(r[b], b, qi) for b in range(B) for qi in range(NQT)]
    jobs.sort(key=lambda x: (-x[0], x[1], x[2]))
    profile = [0] * NSLOT
    assign = [[None] * NSLOT for _ in range(N_CORES)]
    for j in range(NSLOT):
        grp = jobs[N_CORES * j:N_CORES * (j + 1)]
        e = NSLOT - 1 - j                  # execution index (ascending t)
        profile[e] = max(1, grp[0][0])
        for c in range(N_CORES):
            assign[c][e] = (grp[c][1], grp[c][2])
    return tuple(profile), assign, r


def _npair(t):
    return -(-t // 2)


def _chunk_cols(profile, nm):
    e = int(nm[1:-1]) if nm[-1] in "ht" else int(nm[1:])
    if nm[0] == "k":
        np_ = _npair(profile[e])
        split = (e == _visit(profile)[0] and np_ > 2)
        if nm[-1] == "h":
            return 2 * 512 if split else np_ * 512
        return (np_ - 2) * 512
    if nm[0] == "q":
        return NDC * QT
    return profile[e] * VF


def _visit(profile):
    """Slot execution order: ascending t rotated left by one, so the
    start streams tiny DMAs and the LAST slot is the smallest (short
    post-PE tail)."""
    order = sorted(range(NSLOT), key=lambda e: (profile[e], e))
    return order[1:] + order[:1]


def _segments(profile):
    """DMA segments: K+Q on the sync ring, V on the scalar ring (the
    scalar ENGINE also runs exp, so its descriptor writes are deferred
    in the emission schedule, not here).  Blob layout follows this
    order; a multi-chunk segment is stored partition-major over the
    WHOLE segment."""
    v0 = _visit(profile)[0]
    segs = []
    for e in range(NSLOT):
        if e == v0 and _npair(profile[e]) > 2:
            segs.append(("sync", [f"k{e}h", f"q{e}"]))
            segs.append(("sync", [f"k{e}t"]))
        else:
            segs.append(("sync", [f"k{e}h", f"q{e}"]))
        segs.append(("scalar", [f"v{e}"]))
    return segs


def _layout(profile):
    """Blob offsets per chunk, in segment order."""
    segs = _segments(profile)
    off = {}
    o = 0
    for _ring, names in segs:
        for nm in names:
            n = _chunk_cols(profile, nm)
            off[nm] = (o, n)
            o += n
    return segs, off, o


def _build_nc(profile):
    segs, off, tot = _layout(profile)

    nc = bacc.Bacc("TRN2", target_bir_lowering=False, debug=False,
                   num_devices=N_CORES)
    blob_d = nc.declare_dram_parameter("blob", [128 * tot], _BF16,
                                       isOutput=False)
    out_d = nc.declare_dram_parameter("out", [NSLOT, QT, DV], _F16,
                                      isOutput=True)
    tmax = max(profile)

    with tile.TileContext(nc) as tc:
        with (
            tc.tile_pool(name="seg", bufs=1) as seg_pool,
            tc.tile_pool(name="p", bufs=2) as p_pool,
            tc.tile_pool(name="osb", bufs=3) as o_pool,
            tc.tile_pool(name="small", bufs=8) as small_pool,
            tc.tile_pool(name="ps_s", bufs=2, space="PSUM") as ps_s,
            tc.tile_pool(name="ps_o", bufs=4, space="PSUM") as ps_o,
        ):
            tiles = {}

            def load(names, engine):
                lo = off[names[0]][0]
                n = sum(off[nm][1] for nm in names)
                t = seg_pool.tile([128, n], _BF16, tag="+".join(names))
                src = blob_d[128 * lo:128 * (lo + n)].rearrange(
                    "(p n) -> p n", p=128)
                engine.dma_start(out=t, in_=src)
                for nm in names:
                    tiles[nm] = (t, off[nm][0] - lo)

            def chunk_slice(nm, a, b):
                t, o = tiles[nm]
                return t[:, o + a:o + b]

            def kt_slice(e, c, kj):
                pair, half = kj // 2, kj % 2
                col = pair * 512 + c * 256 + half * 128
                hcols = off[f"k{e}h"][1]
                if col < hcols:
                    return chunk_slice(f"k{e}h", col, col + 128)
                return chunk_slice(f"k{e}t", col - hcols, col - hcols + 128)

            def qt_slice(e, c):
                return chunk_slice(f"q{e}", c * QT, (c + 1) * QT)

            def v1_slice(e, kj):
                return chunk_slice(f"v{e}", kj * VF, (kj + 1) * VF)

            kq_segs, v_segs = {}, {}
            for ring, names in segs:
                e = int(names[0][1:-1] if names[0][0] == "k"
                        else names[0][1:])
                if ring == "sync":
                    kq_segs.setdefault(e, []).append(names)
                else:
                    v_segs[e] = names

            visit = _visit(profile)

            def issue_kq(e):
                for names in kq_segs[e]:
                    load(names, nc.sync)

            def issue_v(e):
                load(v_segs[e], nc.scalar)

            # Prologue: only the first slots' segments.  Later segments
            # are issued INSIDE the emission stream (after each slot's
            # first exp) so the scalar engine's descriptor writes never
            # delay the early exp chain, and the sync ring streams K/Q
            # a few slots ahead of consumption.
            for i in range(min(3, NSLOT)):
                issue_kq(visit[i])
            for i in range(min(2, NSLOT)):
                issue_v(visit[i])

            # HAM warm-up: matmuls on an UNINITIALIZED tile (no producer ->
            # no waits -> PE starts right after its preamble) into a
            # throwaway PSUM group.  The tiny DVE read keeps DCE away.
            warm = small_pool.tile([128, QT], _BF16, tag="warm")
            wps = ps_o.tile([128, QT], _F32, tag="o", name="warm_ps")
            for w in range(WARMUP_MMS):
                nc.tensor.matmul(wps, lhsT=warm[:, :128], rhs=warm,
                                 start=(w == 0), stop=(w == WARMUP_MMS - 1))
            nc.vector.tensor_copy(out=warm[:, 0:1], in_=wps[:, 0:1])

            out_engines = [nc.sync, nc.scalar]

            def s2_unit(e, p_sb, s, o_job, out_eng, split_out):
                """One stage-2 q-subtile: accumulate over k tiles,
                normalize, and DMA (batched per job; per-subtile for the
                final slot so the tail DMA overlaps the last chains)."""
                t = profile[e]
                o_ps = ps_o.tile([128, VF], _F32, tag="o")
                for kj in range(t):
                    nc.tensor.matmul(
                        o_ps,
                        lhsT=p_sb[:, kj * QT + s * QS:kj * QT + (s + 1) * QS],
                        rhs=v1_slice(e, kj),
                        start=(kj == 0), stop=(kj == t - 1),
                    )
                recip = small_pool.tile([128, 1], _F32, tag="r")
                nc.vector.reciprocal(out=recip, in_=o_ps[:, DV:DV + 1])
                nc.vector.tensor_scalar_mul(
                    out=o_job[:, s * DV:(s + 1) * DV], in0=o_ps[:, :DV],
                    scalar1=recip)
                if split_out:
                    out_eng.dma_start(
                        out=out_d[e, s * QS:(s + 1) * QS, :],
                        in_=o_job[:, s * DV:(s + 1) * DV])
                elif s == NQS - 1:
                    dst = out_d[e].rearrange("(s p) v -> p s v", s=NQS, p=QS)
                    src = o_job.rearrange("p (s v) -> p s v", s=NQS)
                    out_eng.dma_start(out=dst, in_=src)

            pending = None  # (e, p_sb, o_job, next_subtile, out_eng)
            for i, e in enumerate(visit):
                t = profile[e]
                p_sb = p_pool.tile([128, tmax * QT], _BF16, tag="p")
                for g in range(_npair(t)):
                    w = 2 if 2 * g + 1 < t else 1
                    ps = ps_s.tile([128, 2 * QT], _F32, tag="s")
                    for h in range(w):
                        kj = 2 * g + h
                        for c in range(NDC):
                            nc.tensor.matmul(
                                ps[:, h * QT:(h + 1) * QT],
                                lhsT=kt_slice(e, c, kj),
                                rhs=qt_slice(e, c),
                                start=(c == 0),
                                stop=(c == NDC - 1),
                            )
                    nc.scalar.activation(
                        out=p_sb[:, g * 2 * QT:g * 2 * QT + w * QT],
                        in_=ps[:, :w * QT],
                        func=mybir.ActivationFunctionType.Exp,
                        scale=1.0 / 16.0)
                    if g == 0:
                        if i + 3 < NSLOT:
                            issue_kq(visit[i + 3])
                        if i + 2 < NSLOT:
                            issue_v(visit[i + 2])
                    # keep the PE fed while ScalarE works through exp:
                    # the previous slot's stage-2 subtile chains slot
                    # between stage-1 groups
                    elif pending is not None and pending[3] < NQS:
                        ep, pp, oj, si, oe = pending
                        s2_unit(ep, pp, si, oj, oe, False)
                        pending = (ep, pp, oj, si + 1, oe)
                if pending is not None:
                    ep, pp, oj, si, oe = pending
                    for s in range(si, NQS):
                        s2_unit(ep, pp, s, oj, oe, False)
                o_job = o_pool.tile([128, NQS * DV], _F16, tag="o_job")
                pending = (e, p_sb, o_job, 0, out_engines[i % 2])
            ep, pp, oj, si, oe = pending
            for s in range(si, NQS):
                s2_unit(ep, pp, s, oj, oe, True)

    nc.compile()
    return nc


def _get_nc(profile):
    if profile not in _NC_CACHE:
        _NC_CACHE[profile] = _build_nc(profile)
    return _NC_CACHE[profile]


def _prepare(query, key, value, valid_length):
    query = np.asarray(query, dtype=np.float32)
    key = np.asarray(key, dtype=np.float32)
    value = np.asarray(value, dtype=np.float32)
    vl = np.asarray(valid_length).astype(np.int64)

    profile, assign, r = _schedule(vl)
    segs, off, tot = _layout(profile)

    bf16 = ml_dtypes.bfloat16

    kz = key.copy()
    for b in range(B):
        kz[b, int(vl[b]):, :] = 0.0
    kzT = kz.transpose(0, 2, 1).reshape(B, NDC, 128, LK)
    qT = query.transpose(0, 2, 1).reshape(B, NDC, 128, LQ)
    v1 = np.concatenate(
        [value, np.ones((B, LK, 1), np.float32)], axis=-1)  # [B, LK, VF]
    # suffix sums at tile boundaries: suf[b, m] = sum_{k >= m*128} v1[b, k]
    blk = v1.reshape(B, NKT, KT, VF).sum(axis=2)            # [B, 16, VF]
    suf = np.zeros((B, NKT + 1, VF), np.float32)
    suf[:, :NKT] = blk[:, ::-1].cumsum(axis=1)[:, ::-1]

    def k_chunk(b, t):
        np_ = _npair(t)
        kp = np.zeros((NDC, 128, np_ * 256), np.float32)
        kp[:, :, :t * KT] = kzT[b][:, :, :t * KT]
        arr = kp.reshape(NDC, 128, np_, 256).transpose(1, 2, 0, 3)
        return arr.reshape(128, np_ * 512)

    def q_chunk(b, qi):
        return qT[b][:, :, qi * QT:(qi + 1) * QT] \
            .transpose(1, 0, 2).reshape(128, NDC * QT)

    def v_chunk(b, t):
        vj = v1[b, :t * KT].copy()                          # [t*128, VF]
        if t < NKT:
            # fold the exact masked-suffix correction into the last two
            # rows of the computed range (both masked -> weight exactly 1)
            T = vj[-1] + vj[-2] + suf[b, t]
            hi = T.astype(bf16).astype(np.float32)
            vj[-1] = hi
            vj[-2] = T - hi
        return vj.reshape(t, 128, VF).transpose(1, 0, 2).reshape(128, t * VF)

    blobs = []
    for c in range(N_CORES):
        parts = {}
        for e, t in enumerate(profile):
            b, qi = assign[c][e]
            kc = k_chunk(b, t)
            if f"k{e}t" in off:
                parts[f"k{e}h"] = kc[:, :2 * 512]
                parts[f"k{e}t"] = kc[:, 2 * 512:]
            else:
                parts[f"k{e}h"] = kc
            parts[f"q{e}"] = q_chunk(b, qi)
            parts[f"v{e}"] = v_chunk(b, t)
        flat_segs = []
        for _ring, names in segs:
            seg = (np.concatenate([parts[nm] for nm in names], axis=1)
                   if len(names) > 1 else parts[names[0]])
            flat_segs.append(seg.astype(bf16).reshape(-1))
        flat = np.concatenate(flat_segs)
        assert flat.shape == (128 * tot,)
        blobs.append(flat)
    return blobs, profile, assign


def _run(inputs, trace=False):
    blobs, profile, assign = _prepare(**inputs)
    in_maps = [{"blob": blobs[c]} for c in range(N_CORES)]
    nc = _get_nc(profile)
    res = run_bass_kernel_spmd(nc, in_maps, core_ids=list(range(N_CORES)),
                               trace=trace)
    out = np.empty((B, LQ, DV), np.float32)
    for c in range(N_CORES):
        r_c = np.asarray(res.results[c]["out"], dtype=np.float32)
        for e in range(NSLOT):
            b, qi = assign[c][e]
            out[b, qi * QT:(qi + 1) * QT] = r_c[e]
    return out, res


def kernel(query, key, value, valid_length):
    out, _ = _run(dict(query=query, key=key, value=value,
                       valid_length=valid_length))
    return out
